# revision 12
# baseline (speedup 1.0000x reference)
"""Causal self-attention (B=4, T=2048, C=1024, H=16, D=64) on 8 TRN2 NeuronCores.

Sharding: 8 cores = 4 batches x 2 head-groups (8 heads each). Each core:
  - QKV projection for its (batch, head-group) column slice of w_attn,
    producing qT/kT in head-pair-packed [d, t] layout (head 2i in partitions
    0-63, head 2i+1 in 64-127 of pair tile i) and v in [t, d].
  - Causal attention in scoresT layout (scores^T[k, q] straight off the PE;
    K=64 matmuls via PE quadrant base-partition addressing; causal mask
    applied by accumulating -1000*triu into the scores PSUM with one extra
    128-col matmul, so exp() emits exact zeros and no DVE masking is needed;
    softmax denominators via an appended ones-column on V).
  - Row-sharded output projection -> per-core partial [T, C].
Host sums the two partials per batch and adds b_proj.

All matmul operands are bf16 (1 cycle/row on the PE), accumulation in fp32
PSUM. Scheduling: the PE p-state drops to half clock for ~3us after any idle
gap, so the emission order keeps an explicit virtual-clock model of the PE
and ACT streams and pumps projection work between attention steps exactly
when the ACT (exp) stream would otherwise stall the PE. Inputs are
host-preswizzled to per-partition-contiguous layouts and loaded over both
HWDGE rings (sync + scalar) as one DMA per tensor.
"""

import sys
import types
from collections import deque

import numpy as np

B, T, C, H, D = 4, 2048, 1024, 16, 64
HG = 8            # heads per core
CG = HG * D       # 512 channels per group
NCORES = 8
PE_NS = 1.0 / 2.4      # ns per PE cycle (full p-state)
ACT_NS = 1.0 / 1.2     # ns per ACT lane-cycle
ACT_FIX = 215.0        # measured fixed overhead per ACTIVATE


def _register_ntff_hook():
    """Register the axon NTFF profile hook if the image's antenv lacks it."""
    try:
        import antenv
        if getattr(antenv, "axon_hooks", None) is not None:
            return
        from trn_agent_boot.trn_boot import _ntff_profile_via_ctypes
        hook = _ntff_profile_via_ctypes("/opt/axon/libaxon_pjrt.so")
        mod = types.ModuleType("antenv.axon_hooks")
        mod._hook = hook
        mod.get_axon_ntff_profile_hook = lambda: mod._hook
        mod.set_axon_ntff_profile_hook = lambda h: setattr(mod, "_hook", h)
        sys.modules["antenv.axon_hooks"] = mod
        antenv.axon_hooks = mod
    except Exception:
        pass


_NC_CACHE = {}


def _build():
    import concourse.bacc as bacc
    import concourse.mybir as mybir
    import concourse.tile as tile
    from contextlib import ExitStack

    F32 = mybir.dt.float32
    F32R = mybir.dt.float32r
    BF16 = mybir.dt.bfloat16
    ADD = mybir.AluOpType.add
    MUL = mybir.AluOpType.mult
    EXP = mybir.ActivationFunctionType.Exp
    COPY = mybir.ActivationFunctionType.Copy

    nc = bacc.Bacc(None, target_bir_lowering=False, debug=False)
    xq_d = [nc.dram_tensor(f"xq{q}", [128, 8, 512], BF16, kind="ExternalInput")
            for q in range(4)]
    wqk_d = nc.dram_tensor("wqk", [128, 8, 1024], BF16, kind="ExternalInput")
    wv_d = nc.dram_tensor("wv", [128, 8, 512], BF16, kind="ExternalInput")
    wp_d = nc.dram_tensor("wp", [128, 4, 1024], BF16, kind="ExternalInput")
    bqk_d = nc.dram_tensor("bqk", [128, 8], F32, kind="ExternalInput")
    bv_d = nc.dram_tensor("bv", [1, 512], BF16, kind="ExternalInput")
    cst_d = nc.dram_tensor("cst", [128, 256], BF16, kind="ExternalInput")
    out_d = nc.dram_tensor("out", [T, C], F32, kind="ExternalOutput")

    with tile.TileContext(nc) as tc, ExitStack() as ctx:
        pers = ctx.enter_context(tc.tile_pool(name="pers", bufs=1))

        # Head-pair packed qT/kT: pair tile hp holds head 2hp in partitions
        # 0-63 and head 2hp+1 in 64-127, both in [d, t] layout.
        qT = [pers.tile([128, T], BF16, name=f"qT{i}") for i in range(4)]
        kT = [pers.tile([128, T], BF16, name=f"kT{i}") for i in range(4)]
        # v_aug[p, j, h, 0:64] = v[t=j*128+p, h*64+d]; [..., 64] = 1.0
        v_aug = pers.tile([128, 16, HG, 65], BF16, name="v_aug")
        yT = [pers.tile([128, T], BF16, name=f"yT{i}") for i in range(4)]
        cst = pers.tile([128, 256], BF16, name="cst")
        ones_q = pers.tile([1, 512], BF16, name="ones_q")
        ones_col = pers.tile([1, 64], F32R, name="ones_col")
        bqk_sb = pers.tile([128, 8], F32, name="bqk_sb")
        bv_sb = pers.tile([1, 512], BF16, name="bv_sb")
        wp_sb = pers.tile([128, 4, 1024], BF16, name="wp_sb")

        utri_mask = cst[:, 0:128]
        id128 = cst[:, 128:256]

        att_pool = ctx.enter_context(tc.tile_pool(name="att_pool", bufs=6))
        nrm_pool = ctx.enter_context(tc.tile_pool(name="nrm_pool", bufs=4))
        out_pool = ctx.enter_context(tc.tile_pool(name="out_pool", bufs=3))
        ps_s_pool = ctx.enter_context(
            tc.tile_pool(name="ps_s_pool", bufs=2, space="PSUM"))
        ps_y_pool = ctx.enter_context(
            tc.tile_pool(name="ps_y_pool", bufs=2, space="PSUM"))
        aux_pool = ctx.enter_context(
            tc.tile_pool(name="aux_pool", bufs=2, space="PSUM"))

        # Phase-1 pools (allocated last, released mid-program in reverse).
        wqk_pool = tc.alloc_tile_pool(name="wqk_pool", bufs=1)
        wv_pool = tc.alloc_tile_pool(name="wv_pool", bufs=1)
        xq_pool = tc.alloc_tile_pool(name="xq_pool", bufs=1)
        wqk_sb = wqk_pool.tile([128, 8, 1024], BF16, name="wqk_sb")
        wv_sb = wv_pool.tile([128, 8, 512], BF16, name="wv_sb")
        xq = [xq_pool.tile([128, 8, 512], BF16, name=f"xq{q}") for q in range(4)]

        # bf16/f32r constants staged via f32 memset + rounding copies.
        stage = pers.tile([128, 512], F32, name="stage")
        nc.vector.memset(stage[:], 1.0)
        nc.vector.tensor_copy(ones_q[:], stage[0:1, :])
        nc.vector.tensor_copy(ones_col[:], stage[0:1, 0:64])
        nc.vector.tensor_copy(
            v_aug[:, :, :, 64:65],
            stage[:, 0:128].rearrange("p (j h) -> p j h", j=16))

        # Startup DMAs. SDMA round-robins packets across all queues with
        # pending work (equal bandwidth per active queue), so only the
        # immediately-needed tensors are issued at t0; the rest are issued
        # from the scalar queue behind anchor ops that depend on early
        # compute, which delays their triggers until the first wave drains.
        nc.scalar.dma_start(bqk_sb[:], bqk_d.ap()[:])
        nc.scalar.dma_start(bv_sb[:], bv_d.ap()[:])
        nc.scalar.dma_start(cst[:], cst_d.ap()[:])
        nc.scalar.dma_start(wv_sb[:], wv_d.ap()[:])
        nc.sync.dma_start(xq[0][:], xq_d[0].ap()[:])
        nc.sync.dma_start(xq[1][:], xq_d[1].ap()[:])
        anchor = pers.tile([1, 16], F32, name="anchor")

        def late_dmas(stage_no):
            if stage_no == 0:
                nc.scalar.activation(anchor[:], v_aug[0:1, 0, 0, 0:16], COPY)
                nc.scalar.dma_start(wqk_sb[:], wqk_d.ap()[:])
            else:
                nc.scalar.activation(anchor[:], v_aug[0:1, 4, 0, 0:16], COPY)
                nc.scalar.dma_start(xq[2][:], xq_d[2].ap()[:])
                nc.scalar.dma_start(xq[3][:], xq_d[3].ap()[:])
                nc.scalar.dma_start(wp_sb[:], wp_d.ap()[:])

        # ---------------- virtual clocks + filler pump ----------------
        clk = {"pe": 0.0, "act": 0.0}

        def pe(ns):
            clk["pe"] += ns

        fillers = deque()

        def pump(target):
            while fillers and clk["pe"] < target:
                fillers.popleft()()

        # ---------------- phase 1 units ----------------
        def v_unit(q, tb):
            pv = aux_pool.tile([128, 512], F32, name="pv", tag="aux")
            nc.tensor.matmul(pv[:], ones_q[:, tb * 128:(tb + 1) * 128],
                             bv_sb[:], start=True, stop=False)
            for c in range(8):
                nc.tensor.matmul(
                    pv[:], xq[q][:, c, tb * 128:(tb + 1) * 128],
                    wv_sb[:, c, :], start=False, stop=(c == 7))
            j = q * 4 + tb
            nc.vector.tensor_copy(
                v_aug[:, j, :, 0:64], pv[:].rearrange("p (h d) -> p h d", h=HG))
            pe((8 * 512 + 512) * PE_NS)

        def qk_unit(q, m):
            pqk = aux_pool.tile([128, 512], F32, name="pqk", tag="aux")
            for c in range(8):
                nc.tensor.matmul(
                    pqk[:], wqk_sb[:, c, m * 128:(m + 1) * 128],
                    xq[q][:, c, :], start=(c == 0), stop=(c == 7))
            dst = qT[m] if m < 4 else kT[m - 4]
            nc.vector.tensor_scalar(
                out=dst[:, q * 512:(q + 1) * 512], in0=pqk[:],
                scalar1=bqk_sb[:, m:m + 1], scalar2=None, op0=ADD)
            pe(8 * 512 * PE_NS)

        # ---------------- attention steps ----------------
        def qk_step(h, c2, j, ps_s):
            hp, hh = h // 2, h % 2
            part = slice(64 * hh, 64 * (hh + 1))
            q0 = 1024 * c2
            dead = max(0, (j - 8 * c2) * 128)
            diag = j >= 8 * c2
            kb = kT[hp][part, j * 128:(j + 1) * 128]
            if dead < 512:
                nc.tensor.matmul(ps_s[:, dead:512], kb,
                                 qT[hp][part, q0 + dead:q0 + 512],
                                 start=True, stop=not diag)
                pe((512 - dead) * PE_NS)
                if diag:
                    nc.tensor.matmul(ps_s[:, dead:dead + 128], utri_mask,
                                     id128, start=False, stop=True,
                                     skip_group_check=True)
                    pe(128 * PE_NS)
                nc.tensor.matmul(ps_s[:, 512:1024], kb,
                                 qT[hp][part, q0 + 512:q0 + 1024],
                                 start=True, stop=True)
                pe(512 * PE_NS)
            else:
                lo = dead
                nc.tensor.matmul(ps_s[:, lo:1024], kb,
                                 qT[hp][part, q0 + lo:q0 + 1024],
                                 start=True, stop=not diag)
                pe((1024 - lo) * PE_NS)
                if diag:
                    nc.tensor.matmul(ps_s[:, lo:lo + 128], utri_mask, id128,
                                     start=False, stop=True,
                                     skip_group_check=True)
                    pe(128 * PE_NS)

        def exp_step(c2, j, ps_s):
            dead = max(0, (j - 8 * c2) * 128)
            att_t = att_pool.tile([128, 1024], BF16, tag="att")
            nc.scalar.activation(att_t[:, dead:1024], ps_s[:, dead:1024],
                                 EXP, scale=0.125)
            clk["act"] = (max(clk["act"], clk["pe"] + 150.0)
                          + (1024 - dead) * ACT_NS + ACT_FIX)
            return att_t

        def av_step(h, c2, j, ps_y0, ps_y1, att_t):
            dead = max(0, (j - 8 * c2) * 128)
            va = v_aug[:, j, h, :]
            if j <= 8 * c2 + 3:
                nc.tensor.matmul(ps_y0[:, dead:512], va, att_t[:, dead:512],
                                 start=(j == 0), stop=(j == 8 * c2 + 3))
                pe((512 - dead) * PE_NS)
            lo = max(512, dead)
            nc.tensor.matmul(ps_y1[:, lo - 512:512], va, att_t[:, lo:1024],
                             start=(j == 0), stop=(j == 8 * c2 + 7))
            pe((1024 - lo) * PE_NS)

        def make_norm(h, cch, ps_y):
            """Emit the DVE sums copy now; return a thunk for the rest."""
            sums = nrm_pool.tile([1, 512], F32R, tag="sums")
            nc.vector.tensor_copy(sums[:], ps_y[64:65, :])

            def finish():
                ps_b = aux_pool.tile([64, 512], F32, name="ps_b", tag="aux")
                nc.tensor.matmul(ps_b[:], ones_col[:], sums[:],
                                 start=True, stop=True)
                pe(512 * PE_NS)
                inv = nrm_pool.tile([64, 512], F32, tag="inv")
                nc.vector.reciprocal_approx_fast(inv[:], ps_b[:])
                ct, hh = h // 2, h % 2
                sl = slice(cch * 512, (cch + 1) * 512)
                if hh == 0:
                    nc.vector.tensor_tensor(
                        out=yT[ct][0:64, sl], in0=ps_y[0:64, :], in1=inv[:],
                        op=MUL)
                else:
                    ystg = nrm_pool.tile([64, 512], BF16, tag="ystg")
                    nc.vector.tensor_tensor(
                        out=ystg[:], in0=ps_y[0:64, :], in1=inv[:], op=MUL)
                    nc.sync.dma_start(yT[ct][64:128, sl], ystg[:])
            return finish

        # ---------------- output projection ----------------
        osb = {}

        def proj_unit(tb, ch):
            """Full 4-matmul unit for rows t < 1024 (era-B filler)."""
            if ch == 0:
                osb[tb] = out_pool.tile([128, 1024], F32, name="o_sb",
                                        tag="o_sb")
            pp = aux_pool.tile([128, 512], F32, name="pp", tag="aux")
            for ct in range(4):
                nc.tensor.matmul(
                    pp[:], yT[ct][:, tb * 128:(tb + 1) * 128],
                    wp_sb[:, ct, ch * 512:(ch + 1) * 512],
                    start=(ct == 0), stop=(ct == 3))
            pe(4 * 512 * PE_NS)
            nc.vector.tensor_copy(osb[tb][:, ch * 512:(ch + 1) * 512], pp[:])
            if ch == 1:
                nc.sync.dma_start(
                    out_d.ap()[tb * 128:(tb + 1) * 128, :], osb.pop(tb)[:])

        osb8 = {}

        def proj_pass(tb, ch, ct):
            """Single-ct pass for rows t >= 1024, accumulated in a persistent
            SBUF tile so the first three passes can interleave with era-B
            attention instead of serializing after the last head."""
            pp = aux_pool.tile([128, 512], F32, name="pp", tag="aux")
            nc.tensor.matmul(
                pp[:], yT[ct][:, tb * 128:(tb + 1) * 128],
                wp_sb[:, ct, ch * 512:(ch + 1) * 512], start=True, stop=True)
            pe(512 * PE_NS)
            dst = osb8[tb][:, ch * 512:(ch + 1) * 512]
            if ct == 0:
                nc.vector.tensor_copy(dst, pp[:])
            else:
                nc.vector.tensor_tensor(out=dst, in0=dst, in1=pp[:], op=ADD)
            if ct == 3 and ch == 1:
                nc.sync.dma_start(
                    out_d.ap()[tb * 128:(tb + 1) * 128, :], osb8[tb][:])

        # ---------------- orchestration ----------------
        # Phase-1 lead: quarters 0-1 straight through.
        for tb in range(4):
            v_unit(0, tb)
        late_dmas(0)
        for tb in range(4):
            v_unit(1, tb)
        late_dmas(1)
        for q in range(2):
            for m in range(8):
                qk_unit(q, m)

        # Era-A filler: just the qT halves of quarters 2-3 (needed at the
        # first era-B step); everything else drains at the era boundary.
        for q in range(2, 4):
            for m in range(4):
                fillers.append(lambda q=q, m=m: qk_unit(q, m))

        def attn_era(c2, head_order, on_head_done=None, margin=800.0):
            steps = [(h, j) for h in head_order for j in range(8 * c2 + 8)]
            n = len(steps)
            state = {}     # h -> (ps_y0, ps_y1)
            exp_done = {}  # idx -> act clock when exp(idx) completes
            att_of = {}
            pend = []
            for idx in range(n + 1):
                if idx < n:
                    h, j = steps[idx]
                    if j == 0:
                        state[h] = (
                            ps_y_pool.tile([65, 512], F32, name="ps_y0",
                                           tag="ps_y"),
                            ps_y_pool.tile([65, 512], F32, name="ps_y1",
                                           tag="ps_y"))
                    ps_s = ps_s_pool.tile([128, 1024], F32, name="ps_s",
                                          tag="ps_s")
                    qk_step(h, c2, j, ps_s)
                    att_of[idx] = exp_step(c2, j, ps_s)
                    exp_done[idx] = clk["act"]
                # pending normalize finishers run one slot late (gives the
                # DVE sums copy time to complete before the PE broadcast mm)
                for fin in pend:
                    fin()
                pend = []
                if idx >= 1:
                    ph, pj = steps[idx - 1]
                    pump(exp_done[idx - 1] + margin)
                    y0, y1 = state[ph]
                    av_step(ph, c2, pj, y0, y1, att_of.pop(idx - 1))
                    if pj == 8 * c2 + 7:
                        pend.append(make_norm(ph, 2 * c2, y0))
                        pend.append(make_norm(ph, 2 * c2 + 1, y1))
                        del state[ph]
                        if on_head_done is not None:
                            on_head_done(ph)
            for fin in pend:
                fin()

        # Era A: q < 1024 attention.
        attn_era(0, list(range(HG)))
        # Boundary drain: remaining qT units, then the v/kT halves of
        # quarters 2-3 (needed from era-B step 8 on). PE-contiguous.
        while fillers:
            fillers.popleft()()
        for q in range(2, 4):
            for tb in range(4):
                v_unit(q, tb)
            for m in range(4, 8):
                qk_unit(q, m)
        xq_pool.release()
        wv_pool.release()
        wqk_pool.release()
        osb8_pool = tc.alloc_tile_pool(name="osb8_pool", bufs=1)
        for tb in range(8, 16):
            osb8[tb] = osb8_pool.tile([128, 1024], F32, name=f"osb8_{tb}")

        # Era B: q >= 1024. Filler = output projection rows t < 1024, plus
        # rows t >= 1024 released ct-chunk by ct-chunk as head pairs finish.
        for tb in range(8):
            for ch in range(2):
                fillers.append(lambda tb=tb, ch=ch: proj_unit(tb, ch))

        order_b = [1, 0, 3, 2, 5, 4, 7, 6]
        done_b = []

        def on_head_done(h):
            done_b.append(h)
            comp = set(done_b[:-1])   # one-head safety delay
            for ct in range(3):       # ct3 goes in the tail
                if ct not in on_head_done.rel and {2 * ct, 2 * ct + 1} <= comp:
                    on_head_done.rel.add(ct)
                    for tb in range(8, 16):
                        for ch in range(2):
                            fillers.append(
                                lambda tb=tb, ch=ch, ct=ct:
                                proj_pass(tb, ch, ct))
        on_head_done.rel = set()

        attn_era(1, order_b, on_head_done)
        while fillers:
            fillers.popleft()()

        # Tail: final ct3 passes and output DMAs.
        for tb in range(8, 16):
            for ch in range(2):
                proj_pass(tb, ch, 3)
        osb8_pool.release()

    nc.compile()
    return nc


def _get_nc():
    if "nc" not in _NC_CACHE:
        _register_ntff_hook()
        _NC_CACHE["nc"] = _build()
    return _NC_CACHE["nc"]


def kernel(x, w_attn, b_attn, w_proj, b_proj, _run_kwargs=None):
    import ml_dtypes
    from concourse.bass_utils import run_bass_kernel_spmd

    bf16 = ml_dtypes.bfloat16
    x = np.asarray(x, dtype=np.float32)
    w_attn = np.asarray(w_attn, dtype=np.float32)
    b_attn = np.asarray(b_attn, dtype=np.float32)
    w_proj = np.asarray(w_proj, dtype=np.float32)
    b_proj = np.asarray(b_proj, dtype=np.float32)

    cst = np.concatenate(
        [np.triu(np.ones((128, 128), dtype=np.float32), 1) * (-1000.0),
         np.eye(128, dtype=np.float32)], axis=1).astype(bf16)

    nc = _get_nc()
    in_maps = []
    for core in range(NCORES):
        b, g = divmod(core, 2)
        cs = slice(g * CG, (g + 1) * CG)
        xs = np.ascontiguousarray(
            x[b].T.reshape(8, 128, 4, 512).transpose(1, 2, 0, 3)).astype(bf16)
        wqk = np.concatenate(
            [w_attn[:, cs], w_attn[:, C + g * CG: C + (g + 1) * CG]], axis=1)
        bqk = np.concatenate(
            [b_attn[cs], b_attn[C + g * CG: C + (g + 1) * CG]])
        im = {
            "wqk": np.ascontiguousarray(
                wqk.reshape(8, 128, 1024).transpose(1, 0, 2)).astype(bf16),
            "wv": np.ascontiguousarray(
                w_attn[:, 2 * C + g * CG: 2 * C + (g + 1) * CG]
                .reshape(8, 128, 512).transpose(1, 0, 2)).astype(bf16),
            "wp": np.ascontiguousarray(
                w_proj[cs, :].reshape(4, 128, 1024)
                .transpose(1, 0, 2)).astype(bf16),
            "bqk": np.ascontiguousarray(
                bqk.reshape(8, 128).T).astype(np.float32),
            "bv": b_attn[2 * C + g * CG: 2 * C + (g + 1) * CG]
                .reshape(1, 512).astype(bf16),
            "cst": cst,
        }
        for q in range(4):
            im[f"xq{q}"] = np.ascontiguousarray(xs[:, q]).astype(bf16)
        in_maps.append(im)

    res = run_bass_kernel_spmd(nc, in_maps, core_ids=list(range(NCORES)),
                               **(_run_kwargs or {}))
    out = np.empty((B, T, C), dtype=np.float32)
    for b in range(B):
        out[b] = res.results[2 * b]["out"] + res.results[2 * b + 1]["out"] + b_proj
    if _run_kwargs:
        kernel.last_results = res
    return out
